# revision 2
# baseline (speedup 1.0000x reference)
"""Trainium2 Bass kernel for degree-3 uniform B-spline basis evaluation.

Problem: x (1024, 8192) fp32, knots = linspace(-2, 2, 12) -> out (1024, 8192, 8)
where out[..., i] is the i-th cubic B-spline basis function (Cox-de Boor).

Math. With uniform knots (spacing h), basis i is a shifted cardinal cubic
B-spline: out_i(x) = C((x - knots[0])/h - i), C supported on [0, 4). Writing
a = |(x - knots[0])/h - i - 2| (distance to the support center), C reflects to

    C = relu(2 - a)^3 / 6  -  (2/3) * relu(1 - a)^3

which is numerically clean (all operands O(1), no cancellation) and returns
exact zeros outside the support, matching the reference's indicator-based
recursion even for |x| beyond the grid.

Kernel. The output (32 MB/core fp32) dominates HBM traffic, so the store is
bf16 (rounding of the exact value: elementwise rel err <= 2^-9, far inside
the 2e-2 gate) and the host up-casts to fp32 while unsharding. That halves
DMA time; to match it, the per-channel compute is spread over THREE engines
so each stays ~68us/core:

  ACT   q2 = Abs(x*(k2/h) - c_i*(k2/h))          (one op per channel, 8/tile)
  Pool  q1 = min(q2 * (k1/k2), 2*k1)             (ONE op batched over the 6
                                                  "E" channels: the clamp has
                                                  channel-independent consts)
  DVE   BSPL_E : out = (2k1 - q1)^3 - relu(k2 - q2)^2 * (k2 - q2)
        (8 ALUs, <=8 DVE stages, one instruction per channel)

with k1 = 6^(-1/3), k2 = (2/3)^(1/3) pre-folding the 1/6 and 2/3 weights
into the cubes. The other 2 "B" channels keep the prologue entirely on ACT
(a second Relu op producing rs = k1*relu(2-a)) and use the 7-ALU BSPL_E2
(out = rs^3 - relu(k2-q2)^2*(k2-q2)) so the Pool engine (slowest: ~0.6
efficiency software ALUs) carries only 6 of the 8 channels.

Engine budget per [128, 8192] core-pass (cost-model rates: DVE 0.96 GHz,
ACT 1.2 GHz, Pool 1.2 GHz x 0.6 eff, DMA ~332 GB/s):
  DVE  8 ops/tile  -> 68.3us   ACT 10 ops -> 68.3us
  Pool 6 ch-ops    -> 68.3us   DMA 4 MB in + 16.8 MB out -> ~63us

All channel results go straight into an interleaved [P, F, 8] bf16 SBUF tile
(stride-8 APs) so every store DMA is one fully contiguous transfer.

Sharding: batch-parallel, rows 128*c .. 128*c+127 on core c (8 cores).
"""

import numpy as np

_CACHE = {}

_K1 = float(6.0 ** (-1.0 / 3.0))        # k1^3 = 1/6
_K2 = float((2.0 / 3.0) ** (1.0 / 3.0))  # k2^3 = 2/3

_P = 128          # SBUF partitions = rows per core
_COLS = 8192      # row length
_NB = 8           # basis functions
_F = 1024         # free-dim span per tile / store DMA
_NE = 6           # channels whose relu-clamp runs on Pool ("E" recipe)
_NCORES = 8


def _register_custom_ops():
    import concourse.dve_ops as dve_ops
    from concourse.dve_ops import DveOp
    from concourse.dve_spec import Spec, Src0, Src1, C0, relu, sq, lower
    from concourse.dve_uop import DveOpSpec

    # BSPL_E: in0 = q1 = min(a*k1, 2*k1), in1 = q2 = a*k2, C2(imm2) = 2*k1,
    # C0(s0) = k2.  r = 2k1 - q1 (= k1*relu(2-a), >=0 by the clamp);
    # out = r^3 - relu(k2 - q2)^2 * (k2 - q2).   8 ALUs, depth 5.
    def ref_e(in0, in1, s0, s1, imm2):
        r = imm2 - in0.astype(np.float32)
        p = np.square(r) * r
        w1 = s0 - in1.astype(np.float32)
        return (p - np.square(np.maximum(w1, 0)) * w1).astype(np.float32)

    def body_e(C2):
        r = C2 - Src0
        p = sq(r) * r
        w1 = C0 - Src1
        return p - sq(relu(w1)) * w1

    # BSPL_E2: in0 = rs = k1*relu(2-a) (post-relu from ACT), in1 = q2.
    # out = rs^3 - relu(k2 - q2)^2 * (k2 - q2).   7 ALUs.
    def ref_e2(in0, in1, s0, s1, imm2):
        rs = in0.astype(np.float32)
        p = np.square(rs) * rs
        w1 = s0 - in1.astype(np.float32)
        return (p - np.square(np.maximum(w1, 0)) * w1).astype(np.float32)

    def body_e2():
        p = sq(Src0) * Src0
        w1 = C0 - Src1
        return p - sq(relu(w1)) * w1

    from concourse.dve_spec import C2 as C2node

    ops = {}
    for name, body, ref in (
        ("BSPL_E", body_e(C2node), ref_e),
        ("BSPL_E2", body_e2(), ref_e2),
    ):
        existing = {op.name: op for op in dve_ops.OPS}
        if name in existing:
            ops[name] = existing[name]
            continue
        spec = Spec(body=body, reference=ref)
        shas = {}
        for ver in ("v3", "v4"):
            shas[ver] = DveOpSpec(name=name, uops=lower(spec, ver=ver)).sha(ver)
        op = DveOp(name, spec, subdim=False, uops_sha=shas)
        dve_ops.OPS.append(op)
        dve_ops.CUSTOM_DVE_SPECS[op.name] = op.spec
        row = max(dve_ops._SUB_OPCODE_FOR_NAME.values()) + 1
        assert row < 0x20
        dve_ops._SUB_OPCODE_FOR_NAME[op.name] = row
        ops[name] = op
    return ops["BSPL_E"], ops["BSPL_E2"]


def _build(knot0: float, h: float, passes: int = 1):
    import concourse.bacc as bacc
    import concourse.mybir as mybir
    from concourse import tile

    AF = mybir.ActivationFunctionType
    ALU = mybir.AluOpType
    bspl_e, bspl_e2 = _register_custom_ops()

    nc = bacc.Bacc("TRN2", target_bir_lowering=False, debug=False,
                   num_devices=_NCORES)
    x_ext = nc.declare_dram_parameter("x", [_P, _COLS], mybir.dt.float32,
                                      isOutput=False)
    out_ext = nc.declare_dram_parameter("out", [_P, _COLS * _NB],
                                        mybir.dt.bfloat16, isOutput=True)

    # Channel -> q2 slot: slots 0.._NE-1 are the Pool-clamped "E" channels,
    # slots _NE..7 the ACT-relu "B" channels.  (Which basis index lands in
    # which recipe is arbitrary; keep it contiguous for readable APs.)
    e_ch = list(range(_NE))            # basis indices via BSPL_E
    b_ch = list(range(_NE, _NB))       # basis indices via BSPL_E2

    with tile.TileContext(nc) as tc:
        with tc.tile_pool(name="xin", bufs=2) as xin, \
             tc.tile_pool(name="q2p", bufs=2) as q2p, \
             tc.tile_pool(name="q1p", bufs=2) as q1p, \
             tc.tile_pool(name="rsp", bufs=2) as rsp, \
             tc.tile_pool(name="ilp", bufs=3) as ilp, \
             tc.tile_pool(name="cst", bufs=1) as cst:
            # ACT's float bias operands must live in SBUF as [P, 1] const APs.
            cvals = sorted({-(knot0 + (i + 2) * h) * _K2 / h
                            for i in range(_NB)} | {2.0 * _K1})
            for v in cvals:
                t = cst.tile([_P, 1], mybir.dt.float32, tag=f"c{v}")
                nc.vector.memset(t[:], float(v))
                nc.const_aps.aps[(mybir.dt.float32, float(v))] = t
            nspan = _COLS // _F

            def _prologue(s):
                """Load span s and run its ACT/Pool prologue ops."""
                xs = xin.tile([_P, _F], mybir.dt.float32, tag="x")
                nc.sync.dma_start(xs[:], x_ext[:, s * _F:(s + 1) * _F])
                q2 = q2p.tile([_P, _NB, _F], mybir.dt.float32, tag="q2")
                for slot, i in enumerate(e_ch + b_ch):
                    c_i = knot0 + (i + 2) * h
                    nc.scalar.activation(q2[:, slot, :], xs[:], AF.Abs,
                                         bias=-c_i * _K2 / h, scale=_K2 / h)
                q1 = q1p.tile([_P, _NE, _F], mybir.dt.float32, tag="q1")
                nc.gpsimd.tensor_scalar(
                    q1[:], q2[:, 0:_NE, :], _K1 / _K2, 2.0 * _K1,
                    ALU.mult, ALU.min)
                rs = rsp.tile([_P, _NB - _NE, _F], mybir.dt.float32, tag="rs")
                for b in range(_NB - _NE):
                    nc.scalar.activation(rs[:, b, :], q2[:, _NE + b, :],
                                         AF.Relu, bias=2.0 * _K1,
                                         scale=-_K1 / _K2)
                return q2, q1, rs

            for rep in range(passes):
                # Software-pipelined emission: issue span s+1's load + ACT +
                # Pool prologue before span s's DVE/store work so the Tile
                # scheduler overlaps prologues with the previous span's
                # compute.
                pending = _prologue(0)
                for s in range(nspan):
                    q2, q1, rs = pending
                    if s + 1 < nspan:
                        pending = _prologue(s + 1)
                    il = ilp.tile([_P, _F, _NB], mybir.dt.bfloat16, tag="il")
                    for slot, i in enumerate(e_ch):
                        nc.vector._custom_dve(
                            bspl_e, out=il[:, :, i],
                            in0=q1[:, slot, :], in1=q2[:, slot, :],
                            s0=_K2, imm2=2.0 * _K1)
                    for b, i in enumerate(b_ch):
                        nc.vector._custom_dve(
                            bspl_e2, out=il[:, :, i],
                            in0=rs[:, b, :], in1=q2[:, _NE + b, :],
                            s0=_K2)
                    nc.sync.dma_start(
                        out_ext[:, s * _F * _NB:(s + 1) * _F * _NB],
                        il.rearrange("p f e -> p (f e)"))

    nc.compile()
    return nc


def _numpy_fallback(x, knots):
    """Cox-de Boor on host — only used if knots are not uniform (the
    reference always generates uniform knots; this is a safety net)."""
    te = x[..., None]
    B = ((knots[:-1] <= te) & (te < knots[1:])).astype(np.float32)
    nk = len(knots)
    for k in range(1, 4):
        n = nk - k - 1
        ld = knots[k:k + n] - knots[:n]
        rd = knots[k + 1:k + 1 + n] - knots[1:1 + n]
        left = np.where(ld != 0, (te - knots[:n]) / ld, 0.0) * B[..., :n]
        right = (np.where(rd != 0, (knots[k + 1:k + 1 + n] - te) / rd, 0.0)
                 * B[..., 1:n + 1])
        B = (left + right).astype(np.float32)
    return B[..., :_NB]


def kernel(x: np.ndarray, knots: np.ndarray | None = None, **_ignored):
    from concourse.bass_utils import run_bass_kernel_spmd

    x = np.ascontiguousarray(np.asarray(x, dtype=np.float32))
    if knots is None:
        knots = np.linspace(-2.0, 2.0, 12, dtype=np.float32)
    knots = np.asarray(knots, dtype=np.float32)
    assert x.shape == (_P * _NCORES, _COLS), x.shape
    knot0 = float(knots[0])
    h = float(knots[-1] - knots[0]) / (len(knots) - 1)
    if not np.allclose(np.diff(knots), h, rtol=1e-5, atol=1e-6):
        return _numpy_fallback(x, knots)

    key = (knot0, h)
    if key not in _CACHE:
        _CACHE[key] = _build(knot0, h)
    nc = _CACHE[key]

    in_maps = [{"x": x[c * _P:(c + 1) * _P]} for c in range(_NCORES)]
    res = run_bass_kernel_spmd(nc, in_maps, list(range(_NCORES)))
    out = np.empty((_P * _NCORES, _COLS, _NB), dtype=np.float32)
    for c in range(_NCORES):
        out[c * _P:(c + 1) * _P] = (
            res.results[c]["out"].astype(np.float32).reshape(_P, _COLS, _NB))
    return out


# revision 3
# speedup vs baseline: 4.9702x; 4.9702x over previous
"""Cube-split variant: see kernel.py docstring. Engine split per span:

  ACT   q2_i = Abs(x*(k2/h) - c_i*(k2/h))         (8 ops, fp32 planar)
  DVE   p4c = relu(2*k2 - q2)^3                    (1 op over [P,8,F], 4 ALUs)
        wc  = relu(s*k2 - s*q2)^2 * (s*k2 - s*q2)  (1 op over [P,8,F], 5 ALUs)
  Pool  out4 = p4c - wc   -> bf16                  (1 tensor_tensor op)
  DMA   out4 [P,8,F] -> out_dram[P, 8, COLS] channel-planar (8x2KB runs)

out4 = 4*C exactly (s = 4^(1/3), so all cube weights are exact powers of two
in bf16); the host multiplies by 0.25 (exact) while up-casting and returns a
zero-copy transposed view.
"""

import numpy as np

_CACHE = {}

_K2 = float((2.0 / 3.0) ** (1.0 / 3.0))  # k2^3 = 2/3
_S = float(4.0 ** (1.0 / 3.0))           # s^3 = 4 (= (k2/k1)^3)

_P = 128
_COLS = 8192
_NB = 8
_F = 1024
_NCORES = 8


def _register_custom_ops():
    import concourse.dve_ops as dve_ops
    from concourse.dve_ops import DveOp
    from concourse.dve_spec import Spec, Src0, C0, C1, relu, sq, lower
    from concourse.dve_uop import DveOpSpec

    def _reg(name, body, ref):
        ex = {op.name: op for op in dve_ops.OPS}
        if name in ex:
            return ex[name]
        spec = Spec(body=body, reference=ref)
        shas = {v: DveOpSpec(name=name, uops=lower(spec, ver=v)).sha(v)
                for v in ("v3", "v4")}
        op = DveOp(name, spec, subdim=False, uops_sha=shas)
        dve_ops.OPS.append(op)
        dve_ops.CUSTOM_DVE_SPECS[name] = op.spec
        row = max(dve_ops._SUB_OPCODE_FOR_NAME.values()) + 1
        assert row < 0x20
        dve_ops._SUB_OPCODE_FOR_NAME[name] = row
        return op

    def _ref_cube(in0, in1, s0, s1, imm2):
        r = np.maximum(s0 - in0.astype(np.float32), 0)
        return (np.square(r) * r).astype(np.float32)

    def _body_cube():
        r = relu(C0 - Src0)
        return sq(r) * r

    def _ref_cubew(in0, in1, s0, s1, imm2):
        t = s1 - in0.astype(np.float32) * s0
        return (np.square(np.maximum(t, 0)) * t).astype(np.float32)

    def _body_cubew():
        t = C1 - Src0 * C0
        return sq(relu(t)) * t

    return (_reg("BSPL_CUBE", _body_cube(), _ref_cube),
            _reg("BSPL_CUBEW", _body_cubew(), _ref_cubew))


def _build(knot0: float, h: float, passes: int = 1):
    import concourse.bacc as bacc
    import concourse.mybir as mybir
    from concourse import tile

    AF = mybir.ActivationFunctionType
    ALU = mybir.AluOpType
    bspl_cube, bspl_cubew = _register_custom_ops()

    nc = bacc.Bacc("TRN2", target_bir_lowering=False, debug=False,
                   num_devices=_NCORES)
    x_ext = nc.declare_dram_parameter("x", [_P, _COLS], mybir.dt.float32,
                                      isOutput=False)
    out_ext = nc.declare_dram_parameter("out", [_P, _NB, _COLS],
                                        mybir.dt.bfloat16, isOutput=True)

    with tile.TileContext(nc) as tc:
        with tc.tile_pool(name="xin", bufs=2) as xin, \
             tc.tile_pool(name="q2p", bufs=2) as q2p, \
             tc.tile_pool(name="pcp", bufs=1) as pcp, \
             tc.tile_pool(name="wcp", bufs=1) as wcp, \
             tc.tile_pool(name="outp", bufs=3) as outp, \
             tc.tile_pool(name="cst", bufs=1) as cst:
            cvals = sorted({-(knot0 + (i + 2) * h) * _K2 / h
                            for i in range(_NB)})
            for v in cvals:
                t = cst.tile([_P, 1], mybir.dt.float32, tag=f"c{v}")
                nc.vector.memset(t[:], float(v))
                nc.const_aps.aps[(mybir.dt.float32, float(v))] = t
            nspan = _COLS // _F

            def _prologue(s):
                xs = xin.tile([_P, _F], mybir.dt.float32, tag="x")
                nc.sync.dma_start(xs[:], x_ext[:, s * _F:(s + 1) * _F])
                q2 = q2p.tile([_P, _NB, _F], mybir.dt.float32, tag="q2")
                for i in range(_NB):
                    c_i = knot0 + (i + 2) * h
                    nc.scalar.activation(q2[:, i, :], xs[:], AF.Abs,
                                         bias=-c_i * _K2 / h, scale=_K2 / h)
                return q2

            for rep in range(passes):
                pending = _prologue(0)
                for s in range(nspan):
                    q2 = pending
                    if s + 1 < nspan:
                        pending = _prologue(s + 1)
                    p4c = pcp.tile([_P, _NB, _F], mybir.dt.float32, tag="p4c")
                    wc = wcp.tile([_P, _NB, _F], mybir.dt.float32, tag="wc")
                    nc.vector._custom_dve(bspl_cube, out=p4c[:], in0=q2[:],
                                          s0=2.0 * _K2)
                    nc.vector._custom_dve(bspl_cubew, out=wc[:], in0=q2[:],
                                          s0=_S, s1=_S * _K2)
                    o4 = outp.tile([_P, _NB, _F], mybir.dt.bfloat16, tag="o4")
                    nc.gpsimd.tensor_tensor(o4[:], p4c[:], wc[:],
                                            ALU.subtract)
                    nc.sync.dma_start(out_ext[:, :, s * _F:(s + 1) * _F],
                                      o4[:])

    nc.compile()
    return nc


def _numpy_fallback(x, knots):
    te = x[..., None]
    B = ((knots[:-1] <= te) & (te < knots[1:])).astype(np.float32)
    nk = len(knots)
    for k in range(1, 4):
        n = nk - k - 1
        ld = knots[k:k + n] - knots[:n]
        rd = knots[k + 1:k + 1 + n] - knots[1:1 + n]
        left = np.where(ld != 0, (te - knots[:n]) / ld, 0.0) * B[..., :n]
        right = (np.where(rd != 0, (knots[k + 1:k + 1 + n] - te) / rd, 0.0)
                 * B[..., 1:n + 1])
        B = (left + right).astype(np.float32)
    return B[..., :_NB]


def kernel(x: np.ndarray, knots: np.ndarray | None = None, **_ignored):
    from concourse.bass_utils import run_bass_kernel_spmd

    x = np.ascontiguousarray(np.asarray(x, dtype=np.float32))
    if knots is None:
        knots = np.linspace(-2.0, 2.0, 12, dtype=np.float32)
    knots = np.asarray(knots, dtype=np.float32)
    assert x.shape == (_P * _NCORES, _COLS), x.shape
    knot0 = float(knots[0])
    h = float(knots[-1] - knots[0]) / (len(knots) - 1)
    if not np.allclose(np.diff(knots), h, rtol=1e-5, atol=1e-6):
        return _numpy_fallback(x, knots)

    key = (knot0, h)
    if key not in _CACHE:
        _CACHE[key] = _build(knot0, h)
    nc = _CACHE[key]

    in_maps = [{"x": x[c * _P:(c + 1) * _P]} for c in range(_NCORES)]
    res = run_bass_kernel_spmd(nc, in_maps, list(range(_NCORES)))
    # Assemble channel-planar (rows, NB, COLS), up-cast with the exact 0.25
    # scale (out4 = 4*C with power-of-two weights: bf16 -> fp32 * 0.25 is
    # exact), then return the zero-copy channel-last view.
    planar = np.empty((_P * _NCORES, _NB, _COLS), dtype=np.float32)
    for c in range(_NCORES):
        planar[c * _P:(c + 1) * _P] = res.results[c]["out"].astype(np.float32)
    planar *= 0.25
    return planar.transpose(0, 2, 1)
